# revision 27
# baseline (speedup 1.0000x reference)
"""DEC soft-assignment (student-t, row-normalized) Trainium2 Bass kernel.

q[n,k] = (1 + ||x_n - c_k||^2/alpha)^(-(alpha+1)/2), row-normalized.

Strategy (8 cores, data-parallel over N):
  ||x-c||^2 = ||x||^2 - 2 x.c + ||c||^2 expanded on-chip:
  - centersT [768,512] built once via PE transposes, scaled by -2/alpha.
  - csq row folded into the PSUM accumulation via a 1-partition matmul.
  - per 128-row tile: PE-transpose emb tile (6x128x128) into one PSUM tile,
    6 fp32r matmuls (k=512 free) accumulate -2*cross/a + csq/a,
    ScalarE adds per-row bias (1 + ||x||^2/a) while copying PSUM->SBUF,
    DVE fast reciprocal, ACT row-sum, DVE normalize.
"""

import contextlib
import os
import sys

sys.path.insert(0, "/opt/trn_rl_repo")

import numpy as np

N_CORES = 8
N, D, K = 65536, 768, 512
NC_ROWS = N // N_CORES          # 8192 rows per core
P = 128                         # partitions
N_TILES = NC_ROWS // P          # 64 row tiles per core
D_CHUNKS = D // P               # 6 contraction chunks

_CACHE = {}


def _emit(nc, tc, emb_d, cen_d, out_d, alpha: float, n_rows: int):
    """Emit the per-core program into an open TileContext."""
    import concourse.bass as bass
    import concourse.mybir as mybir
    from concourse.masks import make_identity

    f32 = mybir.dt.float32
    f32r = mybir.dt.float32r
    bf16 = mybir.dt.bfloat16
    ts = bass.ts

    inv_a = 1.0 / alpha
    _SQRT_INV_A = float(inv_a ** 0.5)
    power = (alpha + 1.0) / 2.0
    n_tiles = n_rows // P
    reps = int(os.environ.get("KBENCH_REPS", "1"))

    csq_dram = nc.dram_tensor("csq_scratch", [K], f32)

    with contextlib.ExitStack() as stack:
        const_pool = stack.enter_context(tc.tile_pool(name="const", bufs=1))
        cen_pool = stack.enter_context(tc.tile_pool(name="cent", bufs=1))
        with (
            tc.tile_pool(name="setup", bufs=2) as setup_pool,
            tc.tile_pool(name="setup_ps", bufs=2, space=bass.MemorySpace.PSUM) as setup_ps,
        ):
            identity = const_pool.tile([P, P], f32)
            make_identity(nc, identity[:])
            ones_row_f32 = const_pool.tile([1, P], f32)
            nc.gpsimd.memset(ones_row_f32[:], 1.0)
            ones_row = const_pool.tile([1, P], f32r)
            nc.scalar.copy(ones_row[:], ones_row_f32[:])

            # centersT chunks [d=128, k=512] scaled by -2/alpha, resident.
            cenT = [
                cen_pool.tile([P, K], f32r, tag=f"cenT{j}", name=f"cenT{j}")
                for j in range(D_CHUNKS)
            ]
            csq_cols = const_pool.tile([P, K // P], f32)
            csq_row_f32 = const_pool.tile([1, K], f32)
            csq_row = const_pool.tile([1, K], f32r)

            for t in range(K // P):
                cnat = setup_pool.tile([P, D], f32, tag="cnat")
                nc.sync.dma_start(cnat[:], cen_d[ts(t, P), :])
                # csq for this block of 128 clusters (scaled by 1/alpha)
                scr = setup_pool.tile([P, D], f32, tag="cscr")
                nc.vector.scalar_tensor_tensor(
                    out=scr[:],
                    in0=cnat[:],
                    scalar=inv_a,
                    in1=cnat[:],
                    op0=mybir.AluOpType.mult,
                    op1=mybir.AluOpType.mult,
                    accum_out=csq_cols[:, t : t + 1],
                )
                # transpose the 6 chunks of this block
                tps = setup_ps.tile([P, D], f32, tag="tps")
                for j in range(D_CHUNKS):
                    nc.tensor.transpose(
                        tps[:, ts(j, P)], cnat[:, ts(j, P)], identity[:]
                    )
                for j in range(D_CHUNKS):
                    nc.scalar.mul(
                        cenT[j][:, ts(t, P)], tps[:, ts(j, P)], -2.0 * inv_a
                    )

            # roundtrip csq through DRAM to turn [128,4] cols into a [1,512] row
            nc.sync.dma_start(
                csq_dram.rearrange("(t p) -> p t", p=P), csq_cols[:]
            )
            nc.sync.dma_start(
                csq_row_f32[:], csq_dram.rearrange("(a k) -> a k", a=1)
            )
            nc.scalar.activation(
                csq_row[:],
                csq_row_f32[:],
                mybir.ActivationFunctionType.Identity,
                bias=1.0,
                scale=1.0,
            )

        with (
            tc.tile_pool(name="io_in", bufs=3) as in_pool,
            tc.tile_pool(name="work", bufs=2) as work_pool,
            tc.tile_pool(name="io_out", bufs=3) as out_pool,
            tc.tile_pool(name="tp_ps", bufs=2, space=bass.MemorySpace.PSUM) as tp_ps,
            tc.tile_pool(name="mm_ps", bufs=2, space=bass.MemorySpace.PSUM) as mm_ps,
        ):
            for i in [t for _ in range(reps) for t in range(n_tiles)]:
                emb_nat = in_pool.tile([P, D], f32, tag="emb")
                nc.sync.dma_start(emb_nat[:], emb_d[ts(i, P), :])

                # bias = 1 + ||x||^2/alpha  (per-partition scalar);
                # alternate engines per tile to balance DVE/ACT load
                sq_scr = work_pool.tile([P, D], f32, tag="sqscr")
                xsq_raw = work_pool.tile([P, 1], f32, tag="xsqraw")
                if i % 2 == 0 and os.environ.get("KOPT_XSQ", "dve") == "alt":
                    nc.scalar.activation(
                        sq_scr[:],
                        emb_nat[:],
                        mybir.ActivationFunctionType.Square,
                        scale=_SQRT_INV_A,
                        accum_out=xsq_raw[:],
                    )
                else:
                    nc.vector.scalar_tensor_tensor(
                        out=sq_scr[:],
                        in0=emb_nat[:],
                        scalar=inv_a,
                        in1=emb_nat[:],
                        op0=mybir.AluOpType.mult,
                        op1=mybir.AluOpType.mult,
                        accum_out=xsq_raw[:],
                    )
                # transpose emb tile: 6 x [128,128] -> one PSUM [128,768]
                tps = tp_ps.tile([P, D], f32, tag="tps")
                for j in range(D_CHUNKS):
                    nc.tensor.transpose(
                        tps[:, ts(j, P)], emb_nat[:, ts(j, P)], identity[:]
                    )
                embT = work_pool.tile([P, D], f32r, tag="embT")
                half = D // 2
                nc.scalar.copy(embT[:, :half], tps[:, :half])
                nc.scalar.copy(embT[:, half:], tps[:, half:])

                # PSUM <- csq/alpha - (2/alpha) cross
                ps = mm_ps.tile([P, K], f32, tag="cross")
                nc.tensor.matmul(
                    ps[:],
                    ones_row[:],
                    csq_row[:],
                    start=True,
                    stop=False,
                )
                for j in range(D_CHUNKS):
                    nc.tensor.matmul(
                        ps[:],
                        embT[:, ts(j, P)],
                        cenT[j][:],
                        start=False,
                        stop=(j == D_CHUNKS - 1),
                    )

                # denom = PSUM + bias = 1 + d^2/alpha
                denom = work_pool.tile([P, K], f32, tag="denom")
                nc.scalar.activation(
                    denom[:],
                    ps[:],
                    mybir.ActivationFunctionType.Identity,
                    bias=xsq_raw[:],
                    scale=1.0,
                )

                numer = work_pool.tile([P, K], f32, tag="numer")
                if power == 1.0:
                    nc.vector.reciprocal_approx_fast(out=numer[:], in_=denom[:])
                else:
                    lnd = work_pool.tile([P, K], f32, tag="lnd")
                    nc.scalar.activation(
                        lnd[:], denom[:], mybir.ActivationFunctionType.Ln
                    )
                    nc.scalar.activation(
                        numer[:],
                        lnd[:],
                        mybir.ActivationFunctionType.Exp,
                        scale=-power,
                    )

                rowsum = work_pool.tile([P, 1], f32, tag="rowsum")
                if os.environ.get("KOPT_ROWSUM", "dve") == "act":
                    rs_scr = work_pool.tile([P, K], f32, tag="rsscr")
                    nc.scalar.activation(
                        rs_scr[:],
                        numer[:],
                        mybir.ActivationFunctionType.Identity,
                        accum_out=rowsum[:],
                    )
                else:
                    rs_scr = work_pool.tile([P, K], f32, tag="rsscr")
                    nc.vector.tensor_scalar(
                        out=rs_scr[:],
                        in0=numer[:],
                        scalar1=1.0,
                        scalar2=0.0,
                        op0=mybir.AluOpType.mult,
                        op1=mybir.AluOpType.add,
                        accum_out=rowsum[:],
                    )
                inv_rs = work_pool.tile([P, 1], f32, tag="invrs")
                nc.vector.reciprocal(inv_rs[:], rowsum[:])

                out_t = out_pool.tile([P, K], f32, tag="out")
                if os.environ.get("KOPT_MUL", "dve") == "dve":
                    nc.vector.tensor_scalar_mul(out_t[:], numer[:], inv_rs[:])
                else:
                    nc.gpsimd.tensor_scalar_mul(out_t[:], numer[:], inv_rs[:])
                nc.sync.dma_start(out_d[ts(i, P), :], out_t[:])


def _build_program(alpha: float):
    """Standalone Bacc program (for CoreSim checks)."""
    import concourse.bacc as bacc
    import concourse.mybir as mybir
    import concourse.tile as tile

    f32 = mybir.dt.float32
    nc = bacc.Bacc(None, target_bir_lowering=False, debug=False, num_devices=N_CORES)
    emb_d = nc.declare_dram_parameter("embeddings", [NC_ROWS, D], f32, isOutput=False)
    cen_d = nc.declare_dram_parameter("cluster_centers", [K, D], f32, isOutput=False)
    out_d = nc.declare_dram_parameter("cluster_p", [NC_ROWS, K], f32, isOutput=True)
    with tile.TileContext(nc) as tc:
        _emit(nc, tc, emb_d, cen_d, out_d, alpha, NC_ROWS)
    nc.finalize()
    return nc


def _get_jitted(alpha: float):
    key = (float(alpha), os.environ.get("KBENCH_REPS", "1"))
    if key in _CACHE:
        return _CACHE[key]

    import jax
    from jax.experimental.shard_map import shard_map
    from jax.sharding import Mesh, PartitionSpec as PS

    import concourse.mybir as mybir
    import concourse.tile as tile
    from concourse.bass2jax import bass_jit

    f32 = mybir.dt.float32

    def body(nc, emb, cen):
        out_d = nc.dram_tensor(
            "cluster_p", [NC_ROWS, K], f32, kind="ExternalOutput"
        )
        with tile.TileContext(nc) as tc:
            _emit(nc, tc, emb, cen, out_d, float(alpha), NC_ROWS)
        return out_d

    f = bass_jit(body, num_devices=N_CORES)
    mesh = Mesh(np.asarray(jax.devices()[:N_CORES]), ("core",))
    sharded = shard_map(
        f,
        mesh=mesh,
        in_specs=(PS("core"), PS(None)),
        out_specs=PS("core"),
        check_rep=False,
    )
    jitted = jax.jit(sharded)
    _CACHE[key] = (jitted, mesh)
    return _CACHE[key]


def kernel(embeddings, cluster_centers, alpha):
    emb = np.ascontiguousarray(np.asarray(embeddings, dtype=np.float32))
    cen = np.ascontiguousarray(np.asarray(cluster_centers, dtype=np.float32))
    jitted, _ = _get_jitted(float(alpha))
    try:
        out = jitted(emb, cen)
        return np.asarray(out)
    except Exception:
        # transient device hiccups have been observed; retry once
        import time as _time

        _time.sleep(60)
        out = jitted(emb, cen)
        return np.asarray(out)
